# revision 56
# baseline (speedup 1.0000x reference)
"""MultiHeadAttention Trainium2 Bass kernel (8 cores).

Problem: B=2, S=2048, D=1024, H=16 heads, DK=64, fp32 in/out.
  q/k/v = x @ W* + b*; scores = q k^T / 8; attn = softmax; ctx = attn v;
  out = ctx @ Wo + bo.

Sharding (8 cores): batch (2-way) x head-group (4-way tensor parallel).
Core c handles b = c // 4 and heads [4g, 4g+4), g = c % 4 (d' slice of 256).
Each core computes a partial out [S, D] (contraction over its 256 d' rows of
Wo); the host sums the 4 partials per batch and adds the bias correction
(bv @ Wo + bo).

Data plane is bf16 (x, W, qT/kT/v, attn, ctxT, out partials); all matmul
accumulation in fp32 PSUM, softmax normalization in fp32. bf16 halves DMA
traffic (10 MB/core total) and runs the PE at 1 cyc/row at any tile size.
End-to-end rel err ~5e-3 vs the 2e-2 gate.

On-device layout: "transposed activations". qT/kT [256, S] (d' on
partitions), v natural [S, 256+ones]. Attention per (head, qi-chunk):
  scoresT[kj, qi] = kT^T qT   (PE)
  attnT = exp(scoresT / 8)    (ACT, psum->sbuf, bf16 out)
  ctxT[d'+sums, qi] += v_aug^T attnT  (PE; ones col in v gives row sums)
  ctxT /= sums  (partition_broadcast + reciprocal_approx_fast + DVE mul)
out-proj: out[s, :] = sum_mt ctxT[:, mt, s]^T wo[mt]  (PE), drain, DMA.

Input DMAs are ordered so compute starts ~5us in: wk/wq (mt0 halves) first,
then xT column-blocks of 512 qi interleaved with wv/wk1/wq1/wo. The
projection matmuls drip into the attention step loop between a step's exp
and its ctx matmuls. The tiny softmax-normalization DMAs (sums-row realign +
hp1 ctxT write) go on the Pool/SWDGE queue so they never head-of-line block
the bulk sync-queue transfers.
"""

import numpy as np

B = 2
S = 2048
D = 1024
H = 16
DK = 64
N_CORES = 8
HL = H // 4  # 4 heads per core
DL = HL * DK  # 256 local d'
QC = 1024  # qi chunk for scores/exp (2 heads x 512)
KJT = S // 128  # 16 kj tiles
KT = D // 128  # 8 contraction tiles for projections

_CACHED_NC = None


def _build():
    import concourse.bacc as bacc
    import concourse.mybir as mybir
    import concourse.tile as tile

    f32 = mybir.dt.float32
    bf16 = mybir.dt.bfloat16
    Exp = mybir.ActivationFunctionType.Exp

    nc = bacc.Bacc(None)

    # Host-prepped layouts (see _in_maps):
    #   xt_h[p, k, s] = x[b][s, k*128+p]
    #   wq{mt}/wk{mt}[p, kt*128+j] = W[kt*128+p, mt*128+j] (local col slice)
    #   wv_h[p, kt*256+j] = Wv[kt*128+p, j]
    #   wo_h[p, mt*1024+n] = Wo[local mt*128+p, n]
    xt_h = nc.declare_dram_parameter("xt_h", [128, KT, S], bf16, isOutput=False)
    wkq0 = nc.declare_dram_parameter("wkq0", [128, 2 * KT * 128], bf16, isOutput=False)
    wkq1 = nc.declare_dram_parameter("wkq1", [128, 2 * KT * 128], bf16, isOutput=False)
    wv_h = nc.declare_dram_parameter("wv_h", [128, KT * DL], bf16, isOutput=False)
    wo_h = nc.declare_dram_parameter("wo_h", [128, 2 * D], bf16, isOutput=False)
    bqk = nc.declare_dram_parameter("bqk", [128, 4], f32, isOutput=False)
    # shift identity: id_sh[j+1, j] = 1, row 0 zero -- maps tile rows 1..64
    # of a [65, N] rhs onto psum rows base..base+63
    id_sh = nc.declare_dram_parameter("id_sh", [DK + 1, DK], bf16, isOutput=False)
    out = nc.declare_dram_parameter("out", [S, D], bf16, isOutput=True)

    with tile.TileContext(nc) as tc:
        with (
            tc.tile_pool(name="persist", bufs=1) as persist,
            tc.tile_pool(name="attn", bufs=4) as atp,
            tc.tile_pool(name="norm", bufs=2) as npl,
            tc.tile_pool(name="ob", bufs=4) as obp,
            tc.tile_pool(name="scps", bufs=2, space="PSUM") as scp,
            tc.tile_pool(name="wsps", bufs=2, space="PSUM") as wsp,
            tc.tile_pool(name="cxps", bufs=1, space="PSUM") as cxp,
        ):
            qT_sb = persist.tile([128, 2, S], bf16, tag="qT")
            kT_sb = persist.tile([128, 2, S], bf16, tag="kT")
            # v_aug = [1 | v] per head (ones FIRST): every ctx matmul's psum
            # rows are sums@0 (broadcastable straight from partition 0 -- no
            # realign DMA), ctx@1..64 (shifted into ctxT's halves by cheap
            # id_sh matmuls at psum bases 0 and 64)
            v_sb = persist.tile([128, KJT, HL, DK + 1], bf16, tag="v")
            id_sb = persist.tile([DK + 1, DK], bf16, tag="idsh")
            ctxT_sb = persist.tile([128, 2, S], bf16, tag="ctxT")
            xt_sb = persist.tile([128, KT, S], bf16, tag="xt")
            # [mt, which(0=k,1=q), kt*128+j]
            wkq_sb = persist.tile([128, 2, 2, KT * 128], bf16, tag="wkq")
            wv_sb = persist.tile([128, KT * DL], bf16, tag="wv")
            wo_sb = persist.tile([128, 2 * D], bf16, tag="wo")
            # [bq mt0, bq mt1, bk mt0, bk mt1]
            bqk_sb = persist.tile([128, 4], f32, tag="bqk")
            ones_bf = persist.tile([128, KJT, HL, 1], bf16, tag="ones")

            # Input DMAs, in consumption order. All on the sync queue; each
            # consumer is gated by range-level tile deps, so compute starts
            # as soon as its own slice has landed. First xT block is split in
            # two so the prologue's first matmuls start ~3us earlier.
            nc.sync.dma_start(out=bqk_sb[:], in_=bqk[:])
            nc.sync.dma_start(
                out=wkq_sb[:, 0].rearrange("p a b -> p (a b)"), in_=wkq0[:]
            )
            nc.sync.dma_start(out=xt_sb[:, :, 0:256], in_=xt_h[:, :, 0:256])
            nc.sync.dma_start(out=xt_sb[:, :, 256:512], in_=xt_h[:, :, 256:512])
            nc.sync.dma_start(out=wv_sb[:], in_=wv_h[:])
            nc.sync.dma_start(
                out=xt_sb[:, :, 512:1024], in_=xt_h[:, :, 512:1024]
            )
            nc.sync.dma_start(
                out=xt_sb[:, :, 1024:1536], in_=xt_h[:, :, 1024:1536]
            )
            nc.sync.dma_start(
                out=xt_sb[:, :, 1536:2048], in_=xt_h[:, :, 1536:2048]
            )
            nc.sync.dma_start(
                out=wkq_sb[:, 1].rearrange("p a b -> p (a b)"), in_=wkq1[:]
            )
            nc.sync.dma_start(out=wo_sb[:], in_=wo_h[:])
            nc.sync.dma_start(out=id_sb[:], in_=id_sh[:])

            # PE warm-keeper: dummy matmuls that keep the Tensor engine's
            # p-state ramped while it would otherwise idle (DMA-bound startup
            # and the final-norm window). Results are never read.
            scratch = persist.tile([128, 256], bf16, tag="scratch")
            nc.vector.memset(scratch[:], 0.0)

            def warm(n):
                for _ in range(n):
                    dm = scp.tile([128, QC], f32, tag="sc", name="warm")
                    nc.tensor.matmul(
                        dm[:, 0:256],
                        scratch[:, 0:128],
                        scratch[:],
                        start=True,
                        stop=True,
                    )

            warm(40)

            # Ones column for the row-sum trick ([1 | v], see above).
            nc.vector.memset(ones_bf[:], 1.0)
            nc.vector.tensor_copy(v_sb[:, :, :, 0:1], ones_bf[:])

            def qk_cols(which, mt, s0, s1):
                """Project qT (which=0) / kT (which=1) columns [s0, s1)."""
                dst = qT_sb if which == 0 else kT_sb
                bcol = 2 * which + mt  # bqk cols: bq0 bq1 bk0 bk1
                ns = slice(s0, s1)
                ps = wsp.tile(
                    [128, s1 - s0], f32, tag="ws", name=f"pj{which}{mt}{s0}"
                )
                for kt in range(KT):
                    nc.tensor.matmul(
                        ps[:],
                        wkq_sb[:, mt, 1 - which, kt * 128 : (kt + 1) * 128],
                        xt_sb[:, kt, ns],
                        start=(kt == 0),
                        stop=(kt == KT - 1),
                    )
                nc.vector.tensor_scalar_add(
                    out=dst[:, mt, ns],
                    in0=ps[:],
                    scalar1=bqk_sb[:, bcol : bcol + 1],
                )

            def qk_chunk(which, mt, n):
                qk_cols(which, mt, n * 512, (n + 1) * 512)

            def v_chunk(jt):
                """Project v rows [jt*128, (jt+1)*128) for all 4 heads."""
                js = slice(jt * 128, (jt + 1) * 128)
                ps = wsp.tile([128, DL], f32, tag="ws", name=f"vp{jt}")
                for kt in range(KT):
                    nc.tensor.matmul(
                        ps[:],
                        xt_sb[:, kt, js],
                        wv_sb[:, kt * DL : (kt + 1) * DL],
                        start=(kt == 0),
                        stop=(kt == KT - 1),
                    )
                nc.vector.tensor_copy(
                    v_sb[:, jt, :, 1 : DK + 1],
                    ps[:].rearrange("p (h d) -> p h d", h=HL),
                )

            def out_proj_st(st, c, tail=False):
                """Out rows [s0, s0+128): 2 x (2 matmuls + drain), one DMA."""
                s0 = c * 512 + st * 128
                ob = obp.tile([128, 2, 512], bf16, tag="ob")
                for nt in range(2):
                    op = wsp.tile(
                        [128, 512], f32, tag="ws", name=f"op{c}{st}{nt}"
                    )
                    for mt2 in range(2):
                        nc.tensor.matmul(
                            op[:],
                            ctxT_sb[:, mt2, s0 : s0 + 128],
                            wo_sb[
                                :, mt2 * D + nt * 512 : mt2 * D + (nt + 1) * 512
                            ],
                            start=(mt2 == 0),
                            stop=(mt2 == 1),
                        )
                    if tail and (st + nt) % 2 == 0:
                        nc.scalar.copy(ob[:, nt, :], op[:])  # ACT idle in tail
                    else:
                        nc.vector.tensor_copy(ob[:, nt, :], op[:])
                nc.sync.dma_start(
                    out=out[s0 : s0 + 128, :],
                    in_=ob[:].rearrange("p a b -> p (a b)"),
                )

            # Prologue: just enough for chunk-phase (c=0, mt=0) to start.
            # kT is built in 256-col pieces matched to the xT block arrivals
            # so the first scores fire as early as possible.
            qk_cols(1, 0, 0, 256)  # kT mt0, kj tiles 0-1
            qk_cols(0, 0, 0, 512)  # qT mt0 n0 (first 512 qi)
            qk_cols(1, 0, 256, 512)  # kT mt0, kj tiles 2-3

            # Remaining projection/out-proj work dripped into the attention
            # loop, emitted between a step's exp and its ctx matmuls. The
            # phase-(0,0) drips are placed to match DMA arrival of the xT
            # column blocks and the mt1 weights. PE is in-order, so a drip
            # placed before its data lands stalls everything behind it. A
            # kT piece covering sc-tile kj' must be emitted at a step
            # strictly before kj'-2 (scores are emitted two steps ahead).
            mid = {}
            mid[(0, 0, 1)] = [lambda: qk_cols(1, 0, 512, 768)]
            mid[(0, 0, 2)] = [lambda: qk_cols(1, 0, 768, 1024)]
            mid[(0, 0, 3)] = [lambda: qk_cols(1, 0, 1024, 1280)]
            mid[(0, 0, 5)] = [lambda: qk_cols(1, 0, 1280, 1536)]
            mid[(0, 0, 7)] = [lambda: qk_cols(1, 0, 1536, 1792)]
            mid[(0, 0, 9)] = [lambda: qk_cols(1, 0, 1792, 2048)]
            mid[(0, 0, 11)] = [lambda: qk_chunk(1, 1, 0)]
            mid[(0, 0, 12)] = [lambda: qk_chunk(0, 1, 0)]
            mid[(0, 0, 13)] = [lambda: qk_chunk(1, 1, 1)]
            mid[(0, 0, 14)] = [lambda: qk_chunk(1, 1, 2)]
            mid[(0, 0, 15)] = [lambda: qk_chunk(1, 1, 3)]
            # (0,1): remaining qT chunks for phases (1,*) onward
            mid[(0, 1, 1)] = [lambda: qk_chunk(0, 0, 1)]
            mid[(0, 1, 3)] = [lambda: qk_chunk(0, 1, 1)]
            # later qT chunks, one phase ahead of use
            mid[(1, 0, 1)] = [lambda: qk_chunk(0, 0, 2)]
            mid[(1, 1, 1)] = [lambda: qk_chunk(0, 1, 2)]
            mid[(2, 0, 1)] = [lambda: qk_chunk(0, 0, 3)]
            mid[(2, 1, 1)] = [lambda: qk_chunk(0, 1, 3)]
            # out-proj for chunk c drips into chunk c+1's phases: the mt=0
            # drips sit at kj >= 7 so the norm chain of (c,1) has slack
            # before the first reader; the mt=1 drips go early (kj 3/5) so
            # their out-DMAs clear the sync queue well before the phase-end
            # norm needs it.
            for c in range(3):
                for i in range(4):
                    st = i
                    mt_, kj_ = (0, 7 + 2 * (i % 2)) if i < 2 else (1, 3 + 2 * (i % 2))
                    mid.setdefault((c + 1, mt_, kj_), []).append(
                        lambda st=st, c=c: out_proj_st(st, c)
                    )

            NCH = S // 512  # 4 qi chunks of 512
            steps = [
                (c, mt, kj)
                for c in range(NCH)
                for mt in range(2)
                for kj in range(KJT)
            ]
            sc_t = {}

            def emit_sc(i):
                c, mt, kj = steps[i]
                sc = scp.tile([128, QC], f32, tag="sc", name=f"sc{c}{mt}{kj}")
                col = slice(c * 512, (c + 1) * 512)
                for hp in range(2):
                    hs = slice(64 * hp, 64 * hp + 64)
                    nc.tensor.matmul(
                        sc[:, hp * 512 : (hp + 1) * 512],
                        kT_sb[hs, mt, kj * 128 : (kj + 1) * 128],
                        qT_sb[hs, mt, col],
                        start=True,
                        stop=True,
                    )
                sc_t[i] = sc

            cxh = {}
            emit_sc(0)
            emit_sc(1)
            for i, (c, mt, kj) in enumerate(steps):
                col = slice(c * 512, (c + 1) * 512)
                if kj == 0:
                    cxh[(c, mt)] = [
                        cxp.tile([DK + 1, 512], f32, tag="cx0", name=f"cx{c}{mt}0"),
                        cxp.tile([DK + 1, 512], f32, tag="cx1", name=f"cx{c}{mt}1"),
                    ]
                at = atp.tile([128, QC], bf16, tag="at")
                nc.scalar.activation(at[:], sc_t.pop(i)[:], Exp, scale=0.125)
                # scores two steps ahead, then background work, then ctx --
                # keeps the next exp's input first in PE program order so the
                # dripped matmuls hide in the exp shadow.
                if i + 2 < len(steps):
                    emit_sc(i + 2)
                for th in mid.get((c, mt, kj), ()):
                    th()
                if c == 0 and mt == 0:
                    v_chunk(kj)
                cx0, cx1 = cxh[(c, mt)]
                nc.tensor.matmul(
                    cx0[:],
                    v_sb[:, kj, 2 * mt, :],
                    at[:, 0:512],
                    start=(kj == 0),
                    stop=(kj == KJT - 1),
                )
                nc.tensor.matmul(
                    cx1[:],
                    v_sb[:, kj, 2 * mt + 1, :],
                    at[:, 512:1024],
                    start=(kj == 0),
                    stop=(kj == KJT - 1),
                )
                if kj == KJT - 1:
                    # normalize both heads: ctxT_h = ctx_h / sums_h.
                    # partition_broadcast reads the sums rows straight from
                    # their offset (partition-64) APs -- no realign DMA for
                    # the sums. hp1's normalized half still needs a
                    # partition shift 0..63 -> 64..127; that one tiny copy
                    # rides the Pool/SWDGE queue so it never blocks the sync
                    # queue's bulk transfers.
                    # Norm, fully DMA-free: cx rows are sums@0 (ones-first
                    # v), so the broadcast reads partition 0 directly; the
                    # normalized rows 1..64 are shifted into ctxT's halves
                    # via id_sh matmuls (psum bases 0/64) + one aligned
                    # psum->sbuf drain. For mid norms the PE part is
                    # deferred into the next phase so the in-order PE never
                    # waits on the DVE chain; the final norm runs it inline
                    # behind warm-keeper dummies.
                    last = i == len(steps) - 1
                    if last:
                        warm(30)
                    cxs = npl.tile([DK + 1, 2, 512], f32, tag="cxs")
                    nc.vector.tensor_copy(cxs[:, 0, :], cx0[:])
                    nc.vector.tensor_copy(cxs[:, 1, :], cx1[:])
                    bc = npl.tile([DK + 1, 2, 512], f32, tag="bc")
                    nc.gpsimd.partition_broadcast(bc[:, 0, :], cxs[0:1, 0, :])
                    nc.gpsimd.partition_broadcast(bc[:, 1, :], cxs[0:1, 1, :])
                    rinv = npl.tile([DK + 1, 2, 512], f32, tag="rinv")
                    nc.vector.reciprocal_approx_fast(
                        out=rinv[:, 0, :], in_=bc[:, 0, :]
                    )
                    nc.vector.reciprocal_approx_fast(
                        out=rinv[:, 1, :], in_=bc[:, 1, :]
                    )
                    tmp0 = npl.tile([DK + 1, 512], bf16, tag="tmp0")
                    nc.vector.tensor_mul(tmp0[:], cxs[:, 0, :], rinv[:, 0, :])
                    tmp1 = npl.tile([DK + 1, 512], bf16, tag="tmp1")
                    nc.vector.tensor_mul(tmp1[:], cxs[:, 1, :], rinv[:, 1, :])

                    def shift_out(mt=mt, col=col, tmp0=tmp0, tmp1=tmp1):
                        shp = wsp.tile(
                            [128, 512], f32, tag="ws", name=f"sh{c}{mt}"
                        )
                        nc.tensor.matmul(
                            shp[0:64, :], id_sb[:], tmp0[:],
                            start=True, stop=True,
                        )
                        nc.tensor.matmul(
                            shp[64:128, :], id_sb[:], tmp1[:],
                            start=True, stop=True,
                        )
                        nc.vector.tensor_copy(ctxT_sb[:, mt, col], shp[:])

                    if last:
                        shift_out()
                    else:
                        nc_, nmt = (c, 1) if mt == 0 else (c + 1, 0)
                        mid.setdefault((nc_, nmt, 1), []).append(shift_out)
            # last chunk's out-proj is the unavoidable tail
            for st in range(4):
                out_proj_st(st, NCH - 1, tail=True)

    nc.compile()
    return nc


def _get_nc():
    global _CACHED_NC
    if _CACHED_NC is None:
        _CACHED_NC = _build()
    return _CACHED_NC


def _in_maps(x, Wq, bq, Wk, bk, Wv, bv, Wo, bo):
    import ml_dtypes

    bf = ml_dtypes.bfloat16

    id_sh = np.zeros((DK + 1, DK), dtype=bf)
    for j in range(DK):
        id_sh[j + 1, j] = 1.0

    # xt_h[p, k, s] = x[b][s, k*128+p]
    xt_hs = []
    for b in range(B):
        xT = np.ascontiguousarray(x[b].T.astype(bf))  # [D, S]
        xt_hs.append(np.ascontiguousarray(xT.reshape(KT, 128, S).transpose(1, 0, 2)))

    def w_mt(W, cs, mt):
        # [p, kt*128 + j] = W[kt*128+p, cs.start + mt*128 + j]
        wc = W[:, cs][:, mt * 128 : (mt + 1) * 128].astype(bf)  # [D, 128]
        return np.ascontiguousarray(
            wc.reshape(KT, 128, 128).transpose(1, 0, 2).reshape(128, KT * 128)
        )

    maps = []
    for c in range(N_CORES):
        b, g = c // 4, c % 4
        cs = slice(g * DL, (g + 1) * DL)
        wv_c = Wv[:, cs].astype(bf)  # [D, DL]
        wv_l = np.ascontiguousarray(
            wv_c.reshape(KT, 128, DL).transpose(1, 0, 2).reshape(128, KT * DL)
        )
        wo_c = Wo[cs, :].astype(bf)  # [DL, D]
        wo_l = np.ascontiguousarray(
            wo_c.reshape(2, 128, D).transpose(1, 0, 2).reshape(128, 2 * D)
        )
        bq_l = bq[cs].reshape(2, 128).T  # [128, 2]
        bk_l = bk[cs].reshape(2, 128).T
        bqk_l = np.ascontiguousarray(
            np.concatenate([bq_l, bk_l], axis=1).astype(np.float32)
        )
        maps.append(
            {
                "xt_h": xt_hs[b],
                "wkq0": np.ascontiguousarray(
                    np.concatenate([w_mt(Wk, cs, 0), w_mt(Wq, cs, 0)], axis=1)
                ),
                "wkq1": np.ascontiguousarray(
                    np.concatenate([w_mt(Wk, cs, 1), w_mt(Wq, cs, 1)], axis=1)
                ),
                "wv_h": wv_l,
                "wo_h": wo_l,
                "bqk": bqk_l,
                "id_sh": id_sh,
            }
        )
    return maps


def _assemble(results, bv, Wo, bo):
    corr = (bv.astype(np.float64) @ Wo.astype(np.float64)) + bo.astype(np.float64)
    outs = []
    for b in range(B):
        acc = np.zeros((S, D), dtype=np.float64)
        for g in range(4):
            acc += results[b * 4 + g]["out"].astype(np.float64)
        outs.append((acc + corr).astype(np.float32))
    return np.stack(outs)


def kernel(x, Wq, bq, Wk, bk, Wv, bv, Wo, bo):
    from concourse.bass_utils import run_bass_kernel_spmd

    x = np.asarray(x, dtype=np.float32)
    Wq = np.asarray(Wq, dtype=np.float32)
    Wk = np.asarray(Wk, dtype=np.float32)
    Wv = np.asarray(Wv, dtype=np.float32)
    Wo = np.asarray(Wo, dtype=np.float32)
    bq = np.asarray(bq, dtype=np.float32)
    bk = np.asarray(bk, dtype=np.float32)
    bv = np.asarray(bv, dtype=np.float32)
    bo = np.asarray(bo, dtype=np.float32)

    nc = _get_nc()
    res = run_bass_kernel_spmd(
        nc, _in_maps(x, Wq, bq, Wk, bk, Wv, bv, Wo, bo), core_ids=list(range(N_CORES))
    )
    return _assemble(res.results, bv, Wo, bo)
